# revision 8
# baseline (speedup 1.0000x reference)
"""TRN2 Bass kernel v2 for nn_Attention: causal MHA, one batch element per core.

Structure (all matmul operands bf16, fp32 PSUM accumulation):
  A:  xT = transpose(x)  (DMA transposes, or PE transposes with a_dmaT=False)
  B2: v = xT^T @ W_v (+ ones col per head for softmax denominators)
  S1: per head-pair m: B1(m) (qkT rows m, m+8) interleaved with C(m, j=0)
  S2: per head-pair m: C(m, j=1), with first-half output projection D(qc 0..3)
      interleaved as PE filler; then D(qc 4..7).
Causal dead columns are skipped in scores/exp/AV; the 128-wide diagonal
blocks are masked by a DVE multiply with a precomputed 0/1 triangle tile
(mask_dve=True) instead of gpsimd affine_select. Softmax denominators ride
as a 65th row of the AV matmul; normalization uses a DVE reciprocal and a
K=2 selector-matmul that broadcasts both heads' reciprocals into one PSUM
bank (bcast="mm"), replacing the gpsimd partition_broadcast (bcast="gp").
"""

import os
import sys

for _p in ("/opt/trn_rl_repo", os.path.expanduser("~/.axon_site/_ro/trn_rl_repo")):
    if os.path.isdir(_p) and _p not in sys.path:
        sys.path.insert(0, _p)

from contextlib import ExitStack

import numpy as np

import concourse.bass as bass
import concourse.tile as tile
from concourse import bacc, mybir
from concourse.masks import make_identity

F32 = mybir.dt.float32
BF = mybir.dt.bfloat16

S = 1024
D = 1024
H = 16
P = 128
NQ = 512
SC = S // P   # 8
DC = D // P   # 8
MQK = 2 * D // P  # 16


def build_kernel(niter=1, psS_bufs=2, po_bufs=2, psb1_bufs=2, psY_bufs=2,
                 e_bufs=6, wq_bufs=6, uniform=False, qk_f32=False, phases="full",
                 ldw_share=False, early_wq=True, pb_bcast=True, d_after=(1, 3, 5, 7),
                 og_stage=False, a_dmaT=True, mask_dve=True, bcast="mm"):
    nc = bacc.Bacc("TRN2", target_bir_lowering=False, debug=False, num_devices=8)

    x_ap = nc.dram_tensor("x", [S, D], F32, kind="ExternalInput").ap()
    wqkv_ap = nc.dram_tensor("W_qkv", [D, 3 * D], F32, kind="ExternalInput").ap()
    bqkv_ap = nc.dram_tensor("b_qkv", [3 * D], F32, kind="ExternalInput").ap()
    wout_ap = nc.dram_tensor("W_out", [D, D], F32, kind="ExternalInput").ap()
    bout_ap = nc.dram_tensor("b_out", [D], F32, kind="ExternalInput").ap()
    y_ap = nc.dram_tensor("y", [S, D], F32, kind="ExternalOutput").ap()

    wqkv_r = wqkv_ap.rearrange("(kc p) n -> p kc n", p=P)
    wout_r = wout_ap.rearrange("(kc p) n -> p kc n", p=P)

    QKDT = F32 if qk_f32 else BF

    with tile.TileContext(nc) as tc:
      for _it in range(niter):
        top = ExitStack()
        p_top = top.enter_context(tc.tile_pool(name="p_top", bufs=1))

        if not a_dmaT:
            ident = p_top.tile([P, P], BF)
            make_identity(nc, ident)

        bqkv_sb = p_top.tile([P, 3 * D // P], F32)
        nc.sync.dma_start(bqkv_sb[:], bqkv_ap.rearrange("(m p) -> p m", p=P))

        if mask_dve:
            # 0/1 upper-triangle mask (keep query-col x >= kv-partition p),
            # duplicated along the middle (head idx) dim.
            tri = p_top.tile([P, 2, P], BF)
            nc.gpsimd.memset(tri[:], 1.0)
            nc.gpsimd.affine_select(
                out=tri[:],
                in_=tri[:],
                compare_op=mybir.AluOpType.is_ge,
                fill=0.0,
                base=0,
                pattern=[[0, 2], [1, P]],
                channel_multiplier=-1,
            )
        if bcast == "mm":
            # selector rows for the reciprocal-broadcast matmuls:
            # selE spreads a [1,NQ] row onto partitions 0:64, selO onto 64:128.
            sel = p_top.tile([1, 2, P], BF)
            nc.vector.memset(sel[:], 0.0)
            nc.vector.memset(sel[0:1, 0, 0:64], 1.0)
            nc.vector.memset(sel[0:1, 1, 64:128], 1.0)

        qkT = p_top.tile([P, MQK, S], QKDT)     # rows of [q;k]^T
        xT = p_top.tile([P, DC, S], BF)         # x^T
        attnT = p_top.tile([P, DC, S], BF)      # attn_out^T (normalized)
        v_sb = p_top.tile([P, SC, H * 65], BF)  # [kv-pos, kv-chunk, head*(64 v + 1 ones)]

        # W_qk chunk prefetch pipeline: allocate + DMA all 16 chunks up front;
        # the wq pool's slot recycling paces the prefetch ~wq_bufs chunks ahead.
        use_early_wq = early_wq and phases not in ("a", "ab")
        wqp0 = tc.alloc_tile_pool(name="wq0", bufs=wq_bufs) if use_early_wq else None
        wq_tiles = {}
        if use_early_wq:
            order = []
            for m in range(H // 2):
                order += [m, 8 + m]
            for mm in order:
                wq = wqp0.tile([P, DC, P], BF, tag="wq", name=f"wq_{mm}")
                nc.gpsimd.dma_start(wq[:], wqkv_r[:, :, mm * P : (mm + 1) * P])
                wq_tiles[mm] = wq

        # W_v for phase B2 (released after)
        wvp = tc.alloc_tile_pool(name="wv", bufs=1)
        wv = wvp.tile([P, DC, D], BF)
        nc.gpsimd.dma_start(wv[:], wqkv_r[:, :, 2 * D :])  # f32 -> bf16 cast

        # ---------------- Phase A: xT = transpose(x) -------------------
        if a_dmaT:
            # xbar transpose-DMA straight from the bf16 staging tile
            with tc.tile_pool(name="xload", bufs=3) as xpool:
                for so in range(SC):
                    x_t = xpool.tile([P, D], BF, tag="x")
                    nc.gpsimd.dma_start(x_t[:], x_ap[so * P : (so + 1) * P, :])
                    for dd in range(DC):
                        nc.sync.dma_start(
                            xT[:, dd, so * P : (so + 1) * P],
                            x_t[:, dd * P : (dd + 1) * P],
                            transpose=True,
                        )
        else:
            with tc.tile_pool(name="xload", bufs=3) as xpool, tc.tile_pool(
                name="pst", bufs=4, space="PSUM"
            ) as pst:
                for so in range(SC):
                    x_t = xpool.tile([P, D], BF, tag="x")
                    nc.gpsimd.dma_start(x_t[:], x_ap[so * P : (so + 1) * P, :])
                    for dd in range(DC):
                        ps = pst.tile([P, P], BF, tag="pt")
                        nc.tensor.transpose(
                            ps[:], x_t[:, dd * P : (dd + 1) * P], ident[:]
                        )
                        if dd % 2 == 0:
                            nc.scalar.copy(xT[:, dd, so * P : (so + 1) * P], ps[:])
                        else:
                            nc.vector.tensor_copy(
                                xT[:, dd, so * P : (so + 1) * P], ps[:]
                            )

        if phases == "a":
            with tc.tile_pool(name="dump", bufs=2) as dp:
                for dd in range(DC):
                    t = dp.tile([P, S], F32, tag="o", name=f"dump_{dd}")
                    nc.vector.tensor_copy(t[:], xT[:, dd, :])
                    nc.sync.dma_start(y_ap[dd * P : (dd + 1) * P, :], t[:])
            wvp.release()
            top.close()
            continue

        # ---------------- Phase B2: v = xT^T @ W_v (+ones) -------------
        ones_view = v_sb[:].rearrange("p so (h c) -> p so h c", c=65)[:, :, :, 64]
        nc.vector.tensor_copy(
            ones_view, nc.const_aps.tensor(1.0, list(ones_view.shape), F32)
        )
        p_b2 = tc.alloc_tile_pool(name="p_b2", bufs=1)
        biasv_bc = p_b2.tile([P, D], F32)
        nc.sync.dma_start(biasv_bc[:], bqkv_ap[2 * D :][None, :].to_broadcast((P, D)))
        with tc.tile_pool(name="psb2", bufs=6, space="PSUM") as psb2:
            for so in range(SC):
                pss2 = [
                    psb2.tile([P, NQ], F32, tag="ps2", name=f"ps2_{so}_{nq}")
                    for nq in range(D // NQ)
                ]
                if ldw_share:
                    # one LDWEIGHTS of xT[:, kc, so-chunk] serves both nq halves
                    for kc in range(DC):
                        for nq in range(D // NQ):
                            nc.tensor.matmul(
                                pss2[nq][:],
                                xT[:, kc, so * P : (so + 1) * P],
                                wv[:, kc, nq * NQ : (nq + 1) * NQ],
                                start=(kc == 0),
                                stop=(kc == DC - 1),
                            )
                for nq in range(D // NQ):
                    ps = pss2[nq]
                    if not ldw_share:
                        for kc in range(DC):
                            nc.tensor.matmul(
                                ps[:],
                                xT[:, kc, so * P : (so + 1) * P],
                                wv[:, kc, nq * NQ : (nq + 1) * NQ],
                                start=(kc == 0),
                                stop=(kc == DC - 1),
                            )
                    dest = v_sb[:, so, :].rearrange("p (h c) -> p h c", c=65)[
                        :, 8 * nq : 8 * nq + 8, 0:64
                    ]
                    nc.vector.tensor_tensor(
                        out=dest,
                        in0=ps[:].rearrange("p (h c) -> p h c", c=64),
                        in1=biasv_bc[:, nq * NQ : (nq + 1) * NQ].rearrange(
                            "p (h c) -> p h c", c=64
                        ),
                        op=mybir.AluOpType.add,
                    )
        p_b2.release()
        wvp.release()

        if phases == "ab":
            with tc.tile_pool(name="dump", bufs=2) as dp:
                for so in range(SC):
                    t = dp.tile([P, S], F32, tag="o", name=f"dump_{so}")
                    nc.vector.tensor_copy(t[:], v_sb[:, so, 0:S])
                    nc.sync.dma_start(y_ap[so * P : (so + 1) * P, :], t[:])
            top.close()
            continue

        # ---------------- Sweeps: B1 + attention + D --------------------
        wop = tc.alloc_tile_pool(name="wo", bufs=1)
        wo_half = []
        for half in range(2):
            woh = wop.tile([P, DC, NQ], BF, name=f"wo_{half}", tag=f"wo{half}")
            nc.gpsimd.dma_start(woh[:], wout_r[:, :, half * NQ : (half + 1) * NQ])
            wo_half.append(woh)

        cs = ExitStack()
        wqp = None if use_early_wq else cs.enter_context(
            tc.tile_pool(name="wq", bufs=wq_bufs)
        )
        psS = cs.enter_context(tc.tile_pool(name="psS", bufs=psS_bufs, space="PSUM"))
        psO = cs.enter_context(tc.tile_pool(name="psO", bufs=po_bufs, space="PSUM"))
        epool = cs.enter_context(tc.tile_pool(name="e", bufs=e_bufs))
        ogpool = cs.enter_context(tc.tile_pool(name="og", bufs=3))
        rtpool = cs.enter_context(tc.tile_pool(name="rt", bufs=4))
        drpool = cs.enter_context(tc.tile_pool(name="dr", bufs=2, space="DRAM"))
        bcpool = cs.enter_context(tc.tile_pool(name="bc", bufs=2))
        stpool = cs.enter_context(tc.tile_pool(name="st", bufs=2))
        # allocated last so it can be released (LIFO) before psY is created
        psb1 = tc.alloc_tile_pool(name="psb1", bufs=psb1_bufs, space="PSUM")

        def emit_B1(m):
            for mm in (m, 8 + m):
                if use_early_wq:
                    wq = wq_tiles[mm]
                else:
                    wq = wqp.tile([P, DC, P], BF, tag="wq", name=f"wq_{mm}")
                    nc.gpsimd.dma_start(wq[:], wqkv_r[:, :, mm * P : (mm + 1) * P])
                pss1 = [
                    psb1.tile([P, NQ], F32, tag="psb1", name=f"psb1_{mm}_{nq}")
                    for nq in range(S // NQ)
                ]
                if ldw_share:
                    for kc in range(DC):
                        for nq in range(S // NQ):
                            nc.tensor.matmul(
                                pss1[nq][:],
                                wq[:, kc, :],
                                xT[:, kc, nq * NQ : (nq + 1) * NQ],
                                start=(kc == 0),
                                stop=(kc == DC - 1),
                            )
                for nq in range(S // NQ):
                    ps = pss1[nq]
                    if not ldw_share:
                        for kc in range(DC):
                            nc.tensor.matmul(
                                ps[:],
                                wq[:, kc, :],
                                xT[:, kc, nq * NQ : (nq + 1) * NQ],
                                start=(kc == 0),
                                stop=(kc == DC - 1),
                            )
                    nc.vector.tensor_scalar(
                        out=qkT[:, mm, nq * NQ : (nq + 1) * NQ],
                        in0=ps[:],
                        scalar1=bqkv_sb[:, mm : mm + 1],
                        scalar2=None,
                        op0=mybir.AluOpType.add,
                    )

        def emit_C(m, j):
            nkc = 4 * (j + 1)
            poE = psO.tile([65, NQ], F32, tag="po", name=f"poE_{m}_{j}")
            poO = psO.tile([65, NQ], F32, tag="po", name=f"poO_{m}_{j}")
            for i in range(nkc):
                i_loc = i - 4 * j
                dead = 0 if uniform else max(0, i_loc * P)
                pss = psS.tile([P, 2, NQ], F32, tag="psS", name=f"psS_{m}_{j}_{i}")
                for idx, base in enumerate((0, 64)):
                    nc.tensor.matmul(
                        pss[:, idx, dead:],
                        qkT[base : base + 64, 8 + m, i * P : (i + 1) * P],
                        qkT[base : base + 64, m, j * NQ + dead : (j + 1) * NQ],
                        start=True,
                        stop=True,
                    )
                e = epool.tile([P, 2, NQ], BF, tag="e", name=f"e_{m}_{j}_{i}")
                nc.scalar.activation(
                    e[:, :, dead:],
                    pss[:, :, dead:],
                    mybir.ActivationFunctionType.Exp,
                    scale=0.125,
                )
                if i_loc >= 0:
                    # triangle-mask the 128-wide diagonal block: keep y' >= x
                    if mask_dve:
                        nc.vector.tensor_tensor(
                            out=e[:, :, dead : dead + P],
                            in0=e[:, :, dead : dead + P],
                            in1=tri[:],
                            op=mybir.AluOpType.mult,
                        )
                    else:
                        nc.gpsimd.affine_select(
                            out=e[:, :, dead : dead + P],
                            in_=e[:, :, dead : dead + P],
                            compare_op=mybir.AluOpType.is_ge,
                            fill=0.0,
                            base=0,
                            pattern=[[0, 2], [1, P]],
                            channel_multiplier=-1,
                        )
                for idx, h in enumerate((2 * m, 2 * m + 1)):
                    po = poE if idx == 0 else poO
                    nc.tensor.matmul(
                        po[:, dead:],
                        v_sb[:, i, 65 * h : 65 * h + 65],
                        e[:, idx, dead:],
                        start=(i == 0),
                        stop=(i == nkc - 1),
                    )
            if bcast == "mm":
                # both heads' denominators -> reciprocals -> one K=2 matmul
                # broadcasts them across partitions (rows 0:64 even, 64:128 odd)
                rtE = rtpool.tile([1, NQ], BF, tag="rt", name=f"rtE_{m}_{j}")
                rtO = rtpool.tile([1, NQ], BF, tag="rt", name=f"rtO_{m}_{j}")
                with nc.allow_low_precision(reason="bf16 recip feeds bcast matmul"):
                    nc.vector.reciprocal(rtE[:], poE[64:65, :])
                    nc.vector.reciprocal(rtO[:], poO[64:65, :])
                rbc_t = psS.tile([P, 2, NQ], F32, tag="psS", name=f"rbc_{m}_{j}")
                rbc = rbc_t[:, 0, :]
                nc.tensor.matmul(rbc, sel[0:1, 0, :], rtE[:], start=True, stop=False)
                nc.tensor.matmul(rbc, sel[0:1, 1, :], rtO[:], start=False, stop=True)
                # stage to SBUF: walrus forbids two PSUM reads in one DVE op
                rbc_sb = bcpool.tile([P, NQ], BF, tag="rbc", name=f"rbcsb_{m}_{j}")
                if (m + j) % 2 == 0:
                    nc.vector.tensor_copy(rbc_sb[:], rbc)
                else:
                    nc.scalar.copy(rbc_sb[:], rbc)
                nc.vector.tensor_tensor(
                    out=attnT[0:64, m, j * NQ : (j + 1) * NQ],
                    in0=poE[0:64, :],
                    in1=rbc_sb[0:64, :],
                    op=mybir.AluOpType.mult,
                )
                st = stpool.tile([64, NQ], BF, tag="st", name=f"st_{m}_{j}")
                nc.vector.tensor_tensor(
                    out=st[:],
                    in0=poO[0:64, :],
                    in1=rbc_sb[64:128, :],
                    op=mybir.AluOpType.mult,
                )
                nc.sync.dma_start(
                    attnT[64:128, m, j * NQ : (j + 1) * NQ], st[:]
                )
                return
            for idx in range(2):
                po = poE if idx == 0 else poO
                if og_stage:
                    og = ogpool.tile([65, NQ], F32, tag="og", name=f"og_{m}_{j}_{idx}")
                    nc.vector.tensor_copy(og[:], po[:])
                else:
                    og = po  # normalize straight out of PSUM
                rt = rtpool.tile([1, NQ], F32, tag="rt", name=f"rt_{m}_{j}_{idx}")
                nc.vector.reciprocal(rt[:], og[64:65, :])
                rbc = bcpool.tile([64, NQ], F32, tag="rbc", name=f"rbc_{m}_{j}_{idx}")
                if pb_bcast:
                    nc.gpsimd.partition_broadcast(rbc[:], rt[:], channels=64)
                else:
                    scr = drpool.tile([NQ], F32, tag="scr", name=f"scr_{m}_{j}_{idx}")
                    nc.sync.dma_start(scr[None, :], rt[:])
                    nc.sync.dma_start(rbc[:], scr[None, :].to_broadcast((64, NQ)))
                if idx == 0:
                    nc.vector.tensor_tensor(
                        out=attnT[0:64, m, j * NQ : (j + 1) * NQ],
                        in0=og[0:64, :],
                        in1=rbc[:],
                        op=mybir.AluOpType.mult,
                    )
                else:
                    st = stpool.tile([64, NQ], BF, tag="st", name=f"st_{m}_{j}")
                    nc.vector.tensor_tensor(
                        out=st[:],
                        in0=og[0:64, :],
                        in1=rbc[:],
                        op=mybir.AluOpType.mult,
                    )
                    nc.sync.dma_start(
                        attnT[64:128, m, j * NQ : (j + 1) * NQ], st[:]
                    )

        # sweep 1: B1 interleaved with C(j=0)
        if phases == "b1only":
            for m in range(H // 2):
                emit_B1(m)
            psb1.release()
            with tc.tile_pool(name="dump", bufs=2) as dp:
                for mm in range(8):
                    t = dp.tile([P, S], F32, tag="o", name=f"dump_{mm}")
                    nc.vector.tensor_copy(t[:], qkT[:, mm, :])
                    nc.sync.dma_start(y_ap[mm * P : (mm + 1) * P, :], t[:])
            cs.close()
            wop.release()
            if use_early_wq:
                wqp0.release()
            top.close()
            continue
        for m in range(H // 2):
            emit_B1(m)
            emit_C(m, 0)
        psb1.release()

        if phases == "abs1":
            with tc.tile_pool(name="dump", bufs=2) as dp:
                for dd in range(DC):
                    t = dp.tile([P, NQ], F32, tag="o", name=f"dump_{dd}")
                    nc.vector.tensor_copy(t[:], attnT[:, dd, 0:NQ])
                    nc.sync.dma_start(y_ap[dd * P : (dd + 1) * P, 0:NQ], t[:])
            cs.close()
            wop.release()
            if use_early_wq:
                wqp0.release()
            top.close()
            continue

        # D setup
        psY = cs.enter_context(tc.tile_pool(name="psY", bufs=psY_bufs, space="PSUM"))
        ypool = cs.enter_context(tc.tile_pool(name="yp", bufs=3))
        boutbc = ypool.tile([P, D], F32, name="boutbc", tag="boutbc")
        nc.sync.dma_start(boutbc[:], bout_ap[None, :].to_broadcast((P, D)))

        def emit_D(qc):
            psy = [
                psY.tile([P, NQ], F32, tag="psY", name=f"psY_{qc}_{nqq}")
                for nqq in range(D // NQ)
            ]
            if ldw_share:
                for kc in range(DC):
                    for nqq in range(D // NQ):
                        nc.tensor.matmul(
                            psy[nqq][:],
                            attnT[:, kc, qc * P : (qc + 1) * P],
                            wo_half[nqq][:, kc, :],
                            start=(kc == 0),
                            stop=(kc == DC - 1),
                        )
            for nqq in range(D // NQ):
                ps = psy[nqq]
                if not ldw_share:
                    for kc in range(DC):
                        nc.tensor.matmul(
                            ps[:],
                            attnT[:, kc, qc * P : (qc + 1) * P],
                            wo_half[nqq][:, kc, :],
                            start=(kc == 0),
                            stop=(kc == DC - 1),
                        )
                yt = ypool.tile([P, NQ], F32, tag="y", name=f"y_{qc}_{nqq}")
                nc.vector.tensor_tensor(
                    out=yt[:],
                    in0=ps[:],
                    in1=boutbc[:, nqq * NQ : (nqq + 1) * NQ],
                    op=mybir.AluOpType.add,
                )
                nc.sync.dma_start(
                    y_ap[qc * P : (qc + 1) * P, nqq * NQ : (nqq + 1) * NQ], yt[:]
                )

        # sweep 2: C(j=1) with first-half D interleaved as PE filler
        nqc = 0
        for m in range(H // 2):
            emit_C(m, 1)
            if m in d_after:
                emit_D(nqc)
                nqc += 1
        for qc in range(nqc, SC):
            emit_D(qc)

        cs.close()
        wop.release()
        if use_early_wq:
            wqp0.release()
        top.close()

    nc.compile()
    return nc


_CACHED = {}


def _get_nc():
    if "nc" not in _CACHED:
        _CACHED["nc"] = build_kernel(niter=1)
    return _CACHED["nc"]


def kernel(x, W_qkv, b_qkv, W_out, b_out):
    x = np.ascontiguousarray(np.asarray(x, dtype=np.float32))
    W_qkv = np.ascontiguousarray(np.asarray(W_qkv, dtype=np.float32))
    b_qkv = np.ascontiguousarray(np.asarray(b_qkv, dtype=np.float32))
    W_out = np.ascontiguousarray(np.asarray(W_out, dtype=np.float32))
    b_out = np.ascontiguousarray(np.asarray(b_out, dtype=np.float32))
    B = x.shape[0]
    assert x.shape == (8, S, D), f"expected x [8, {S}, {D}], got {x.shape}"

    from concourse.bass_utils import run_bass_kernel_spmd

    nc = _get_nc()
    in_maps = [
        {
            "x": np.ascontiguousarray(x[b]),
            "W_qkv": W_qkv,
            "b_qkv": b_qkv,
            "W_out": W_out,
            "b_out": b_out,
        }
        for b in range(B)
    ]
    res = run_bass_kernel_spmd(nc, in_maps, list(range(B)))
    return np.stack([res.results[b]["y"] for b in range(B)]).astype(np.float32)


# revision 26
# speedup vs baseline: 4.5404x; 4.5404x over previous
"""TRN2 Bass kernel v5 for nn_Attention: causal MHA, one batch element per core.

Structure (all matmul operands bf16, fp32 PSUM accumulation):
  A:  xT = transpose(x) via PE transposes + scalar/vector PSUM->SBUF copies
      (a_dmaT=True uses xbar transpose-DMAs instead: measured much slower)
  B2: v = xT^T @ W_v (+ ones col per head for softmax denominators),
      nq-major so it can start on the first W_v half-DMA
  S1: per head-pair m: C(m, j=0) with B1(m+1) (qkT rows) software-pipelined
      into the scores->exp->AV latency slots
  S2: per head-pair m: C(m, j=1) with output projection D(qc 0..3) as PE
      filler (D(qc<4) only needs sweep-1 attnT); D(qc 4..7) drains after.

Key scheduling facts learned on HW:
 - The PE queue is in-order: an AV matmul waiting on its exp blocks all
   later (ready) matmuls, so emit_C runs scores one i-unit ahead of AV and
   pumps filler matmul generators (B1/D) between them.
 - The two scores matmuls per i-unit (heads 2m / 2m+1, K=64 at base
   partitions 0/64) auto-derive tile_position row groups and run
   concurrently on HW.
 - DMA queue order matters: x staging loads are emitted on the Pool queue
   before W_v before the 16 W_qk prefetch chunks.
Causal dead columns are skipped in scores/exp/AV; the 128-wide diagonal
blocks are masked by a DVE multiply with a precomputed 0/1 triangle tile.
Softmax denominators ride as a 65th row of the AV matmul; normalization is
a DVE reciprocal + gpsimd partition_broadcast; the odd head's normalize
writes attnT[64:128] directly (st_direct, cross-base DVE op verified on HW).
"""

import os
import sys

for _p in ("/opt/trn_rl_repo", os.path.expanduser("~/.axon_site/_ro/trn_rl_repo")):
    if os.path.isdir(_p) and _p not in sys.path:
        sys.path.insert(0, _p)

from contextlib import ExitStack

import numpy as np

import concourse.bass as bass
import concourse.tile as tile
from concourse import bacc, mybir
from concourse.masks import make_identity

F32 = mybir.dt.float32
BF = mybir.dt.bfloat16

S = 1024
D = 1024
H = 16
P = 128
NQ = 512
SC = S // P   # 8
DC = D // P   # 8
MQK = 2 * D // P  # 16


def build_kernel(niter=1, psS_bufs=2, po_bufs=2, psb1_bufs=2, psY_bufs=2,
                 e_bufs=10, wq_bufs=6, uniform=False, qk_f32=False, phases="full",
                 ldw_share=False, early_wq=True, pb_bcast=True, d_after=(0, 1, 2, 3),
                 og_stage=False, a_dmaT=False, mask_dve=True, bcast="gp", f1=4, f2=0.5,
                 persist_x=True, st_direct=True):
    nc = bacc.Bacc("TRN2", target_bir_lowering=False, debug=False, num_devices=8)

    x_ap = nc.dram_tensor("x", [S, D], F32, kind="ExternalInput").ap()
    wqkv_ap = nc.dram_tensor("W_qkv", [D, 3 * D], F32, kind="ExternalInput").ap()
    bqkv_ap = nc.dram_tensor("b_qkv", [3 * D], F32, kind="ExternalInput").ap()
    wout_ap = nc.dram_tensor("W_out", [D, D], F32, kind="ExternalInput").ap()
    bout_ap = nc.dram_tensor("b_out", [D], F32, kind="ExternalInput").ap()
    y_ap = nc.dram_tensor("y", [S, D], F32, kind="ExternalOutput").ap()

    wqkv_r = wqkv_ap.rearrange("(kc p) n -> p kc n", p=P)
    wout_r = wout_ap.rearrange("(kc p) n -> p kc n", p=P)

    QKDT = F32 if qk_f32 else BF

    with tile.TileContext(nc) as tc:
      if persist_x:
          # cross-iteration double-buffered x staging + xT so the next
          # iteration's phase A overlaps this iteration's sweeps
          xT_pool = tc.alloc_tile_pool(name="xTp", bufs=2)
          xpool_p = tc.alloc_tile_pool(name="xloadp", bufs=2 * SC)
      for _it in range(niter):
        top = ExitStack()
        p_top = top.enter_context(tc.tile_pool(name="p_top", bufs=1))

        if not a_dmaT:
            ident = p_top.tile([P, P], BF)
            make_identity(nc, ident)

        bqkv_sb = p_top.tile([P, 3 * D // P], F32)
        nc.sync.dma_start(bqkv_sb[:], bqkv_ap.rearrange("(m p) -> p m", p=P))

        if mask_dve:
            # 0/1 upper-triangle mask (keep query-col x >= kv-partition p),
            # duplicated along the middle (head idx) dim.
            tri = p_top.tile([P, 2, P], BF)
            nc.gpsimd.memset(tri[:], 1.0)
            nc.gpsimd.affine_select(
                out=tri[:],
                in_=tri[:],
                compare_op=mybir.AluOpType.is_ge,
                fill=0.0,
                base=0,
                pattern=[[0, 2], [1, P]],
                channel_multiplier=-1,
            )
        if bcast == "mm":
            # selector rows for the reciprocal-broadcast matmuls:
            # selE spreads a [1,NQ] row onto partitions 0:64, selO onto 64:128.
            sel = p_top.tile([1, 2, P], BF)
            nc.vector.memset(sel[:], 0.0)
            nc.vector.memset(sel[0:1, 0, 0:64], 1.0)
            nc.vector.memset(sel[0:1, 1, 64:128], 1.0)

        qkT = p_top.tile([P, MQK, S], QKDT)     # rows of [q;k]^T
        if persist_x:
            xT = xT_pool.tile([P, DC, S], BF, tag="xT", name=f"xT_{_it}")
        else:
            xT = p_top.tile([P, DC, S], BF)     # x^T
        attnT = p_top.tile([P, DC, S], BF)      # attn_out^T (normalized)
        v_sb = p_top.tile([P, SC, H * 65], BF)  # [kv-pos, kv-chunk, head*(64 v + 1 ones)]

        # Pools allocated in LIFO-release order (wq0 released last, then wv,
        # then xload right after phase A), but the Pool-queue DMAs are emitted
        # in priority order: x staging (phase A) first, then W_v (B2), then
        # the W_qk prefetch chunks (B1, paced by wq pool slot recycling).
        use_early_wq = early_wq and phases not in ("a", "ab")
        wqp0 = tc.alloc_tile_pool(name="wq0", bufs=wq_bufs) if use_early_wq else None
        wvp = tc.alloc_tile_pool(name="wv", bufs=1)
        xpool = xpool_p if persist_x else tc.alloc_tile_pool(name="xload", bufs=SC)

        x_stage = []
        for so in range(SC):
            x_t = xpool.tile([P, D], BF, tag="x", name=f"x_t_{_it}_{so}")
            nc.gpsimd.dma_start(x_t[:], x_ap[so * P : (so + 1) * P, :])
            x_stage.append(x_t)

        # W_v in halves so B2 (nq-major) can start on the first half
        wv = wvp.tile([P, DC, D], BF)
        for nqh in range(2):
            nc.gpsimd.dma_start(
                wv[:, :, nqh * NQ : (nqh + 1) * NQ],
                wqkv_r[:, :, 2 * D + nqh * NQ : 2 * D + (nqh + 1) * NQ],
            )

        wq_tiles = {}
        if use_early_wq:
            order = []
            for m in range(H // 2):
                order += [m, 8 + m]
            for mm in order:
                wq = wqp0.tile([P, DC, P], BF, tag="wq", name=f"wq_{mm}")
                nc.gpsimd.dma_start(wq[:], wqkv_r[:, :, mm * P : (mm + 1) * P])
                wq_tiles[mm] = wq

        # ---------------- Phase A: xT = transpose(x) -------------------
        if a_dmaT:
            # xbar transpose-DMA straight from the bf16 staging tile
            for so in range(SC):
                x_t = x_stage[so]
                for dd in range(DC):
                    nc.sync.dma_start(
                        xT[:, dd, so * P : (so + 1) * P],
                        x_t[:, dd * P : (dd + 1) * P],
                        transpose=True,
                    )
        else:
            with tc.tile_pool(name="pst", bufs=4, space="PSUM") as pst:
                for so in range(SC):
                    x_t = x_stage[so]
                    for dd in range(DC):
                        ps = pst.tile([P, P], BF, tag="pt")
                        nc.tensor.transpose(
                            ps[:], x_t[:, dd * P : (dd + 1) * P], ident[:]
                        )
                        if dd % 2 == 0:
                            nc.scalar.copy(xT[:, dd, so * P : (so + 1) * P], ps[:])
                        else:
                            nc.vector.tensor_copy(
                                xT[:, dd, so * P : (so + 1) * P], ps[:]
                            )
        if not persist_x:
            xpool.release()

        if phases == "a":
            with tc.tile_pool(name="dump", bufs=2) as dp:
                for dd in range(DC):
                    t = dp.tile([P, S], F32, tag="o", name=f"dump_{dd}")
                    nc.vector.tensor_copy(t[:], xT[:, dd, :])
                    nc.sync.dma_start(y_ap[dd * P : (dd + 1) * P, :], t[:])
            wvp.release()
            top.close()
            continue

        # ---------------- Phase B2: v = xT^T @ W_v (+ones) -------------
        ones_view = v_sb[:].rearrange("p so (h c) -> p so h c", c=65)[:, :, :, 64]
        nc.vector.tensor_copy(
            ones_view, nc.const_aps.tensor(1.0, list(ones_view.shape), F32)
        )
        p_b2 = tc.alloc_tile_pool(name="p_b2", bufs=1)
        biasv_bc = p_b2.tile([P, D], F32)
        nc.sync.dma_start(biasv_bc[:], bqkv_ap[2 * D :][None, :].to_broadcast((P, D)))
        with tc.tile_pool(name="psb2", bufs=6, space="PSUM") as psb2:
            for nq in range(D // NQ):
                for so in range(SC):
                    ps = psb2.tile([P, NQ], F32, tag="ps2", name=f"ps2_{so}_{nq}")
                    for kc in range(DC):
                        nc.tensor.matmul(
                            ps[:],
                            xT[:, kc, so * P : (so + 1) * P],
                            wv[:, kc, nq * NQ : (nq + 1) * NQ],
                            start=(kc == 0),
                            stop=(kc == DC - 1),
                        )
                    dest = v_sb[:, so, :].rearrange("p (h c) -> p h c", c=65)[
                        :, 8 * nq : 8 * nq + 8, 0:64
                    ]
                    nc.vector.tensor_tensor(
                        out=dest,
                        in0=ps[:].rearrange("p (h c) -> p h c", c=64),
                        in1=biasv_bc[:, nq * NQ : (nq + 1) * NQ].rearrange(
                            "p (h c) -> p h c", c=64
                        ),
                        op=mybir.AluOpType.add,
                    )
        p_b2.release()
        wvp.release()

        if phases == "ab":
            with tc.tile_pool(name="dump", bufs=2) as dp:
                for so in range(SC):
                    t = dp.tile([P, S], F32, tag="o", name=f"dump_{so}")
                    nc.vector.tensor_copy(t[:], v_sb[:, so, 0:S])
                    nc.sync.dma_start(y_ap[so * P : (so + 1) * P, :], t[:])
            top.close()
            continue

        # ---------------- Sweeps: B1 + attention + D --------------------
        wop = tc.alloc_tile_pool(name="wo", bufs=1)
        wo_half = []
        for half in range(2):
            woh = wop.tile([P, DC, NQ], BF, name=f"wo_{half}", tag=f"wo{half}")
            nc.gpsimd.dma_start(woh[:], wout_r[:, :, half * NQ : (half + 1) * NQ])
            wo_half.append(woh)

        cs = ExitStack()
        wqp = None if use_early_wq else cs.enter_context(
            tc.tile_pool(name="wq", bufs=wq_bufs)
        )
        psS = cs.enter_context(tc.tile_pool(name="psS", bufs=psS_bufs, space="PSUM"))
        psO = cs.enter_context(tc.tile_pool(name="psO", bufs=po_bufs, space="PSUM"))
        epool = cs.enter_context(tc.tile_pool(name="e", bufs=e_bufs))
        ogpool = cs.enter_context(tc.tile_pool(name="og", bufs=3))
        rtpool = cs.enter_context(tc.tile_pool(name="rt", bufs=4))
        drpool = cs.enter_context(tc.tile_pool(name="dr", bufs=2, space="DRAM"))
        bcpool = cs.enter_context(tc.tile_pool(name="bc", bufs=2))
        stpool = cs.enter_context(tc.tile_pool(name="st", bufs=2))
        # allocated last so it can be released (LIFO) before psY is created
        psb1 = tc.alloc_tile_pool(name="psb1", bufs=psb1_bufs, space="PSUM")

        def b1_gen(m):
            """B1 for head-pair m as a generator; yields at 2-matmul chunk
            boundaries so the driver can interleave it into C's PE stream."""
            for mm in (m, 8 + m):
                if use_early_wq:
                    wq = wq_tiles[mm]
                else:
                    wq = wqp.tile([P, DC, P], BF, tag="wq", name=f"wq_{mm}")
                    nc.gpsimd.dma_start(wq[:], wqkv_r[:, :, mm * P : (mm + 1) * P])
                for nq in range(S // NQ):
                    ps = psb1.tile([P, NQ], F32, tag="psb1", name=f"psb1_{mm}_{nq}")
                    for kc0 in range(0, DC, 2):
                        for kc in (kc0, kc0 + 1):
                            nc.tensor.matmul(
                                ps[:],
                                wq[:, kc, :],
                                xT[:, kc, nq * NQ : (nq + 1) * NQ],
                                start=(kc == 0),
                                stop=(kc == DC - 1),
                            )
                        yield
                    nc.vector.tensor_scalar(
                        out=qkT[:, mm, nq * NQ : (nq + 1) * NQ],
                        in0=ps[:],
                        scalar1=bqkv_sb[:, mm : mm + 1],
                        scalar2=None,
                        op0=mybir.AluOpType.add,
                    )

        fill_q = []

        def pump(n):
            done = 0
            while fill_q and done < n:
                try:
                    next(fill_q[0])
                    done += 1
                except StopIteration:
                    fill_q.pop(0)

        def emit_C(m, j, fill_per_unit=0):
            nkc = 4 * (j + 1)
            poE = psO.tile([65, NQ], F32, tag="po", name=f"poE_{m}_{j}")
            poO = psO.tile([65, NQ], F32, tag="po", name=f"poO_{m}_{j}")
            es = {}

            def S(i):
                i_loc = i - 4 * j
                dead = 0 if uniform else max(0, i_loc * P)
                pss = psS.tile([P, 2, NQ], F32, tag="psS", name=f"psS_{m}_{j}_{i}")
                for idx, base in enumerate((0, 64)):
                    nc.tensor.matmul(
                        pss[:, idx, dead:],
                        qkT[base : base + 64, 8 + m, i * P : (i + 1) * P],
                        qkT[base : base + 64, m, j * NQ + dead : (j + 1) * NQ],
                        start=True,
                        stop=True,
                    )
                e = epool.tile([P, 2, NQ], BF, tag="e", name=f"e_{m}_{j}_{i}")
                nc.scalar.activation(
                    e[:, :, dead:],
                    pss[:, :, dead:],
                    mybir.ActivationFunctionType.Exp,
                    scale=0.125,
                )
                if i_loc >= 0:
                    # triangle-mask the 128-wide diagonal block: keep y' >= x
                    if mask_dve:
                        nc.vector.tensor_tensor(
                            out=e[:, :, dead : dead + P],
                            in0=e[:, :, dead : dead + P],
                            in1=tri[:],
                            op=mybir.AluOpType.mult,
                        )
                    else:
                        nc.gpsimd.affine_select(
                            out=e[:, :, dead : dead + P],
                            in_=e[:, :, dead : dead + P],
                            compare_op=mybir.AluOpType.is_ge,
                            fill=0.0,
                            base=0,
                            pattern=[[0, 2], [1, P]],
                            channel_multiplier=-1,
                        )
                es[i] = e

            S(0)
            fill_acc = 0.0
            for i in range(nkc):
                if i + 1 < nkc:
                    S(i + 1)
                fill_acc += fill_per_unit
                if fill_acc >= 1.0:
                    n_f = int(fill_acc)
                    fill_acc -= n_f
                    pump(n_f)
                i_loc = i - 4 * j
                dead = 0 if uniform else max(0, i_loc * P)
                e = es.pop(i)
                for idx, h in enumerate((2 * m, 2 * m + 1)):
                    po = poE if idx == 0 else poO
                    nc.tensor.matmul(
                        po[:, dead:],
                        v_sb[:, i, 65 * h : 65 * h + 65],
                        e[:, idx, dead:],
                        start=(i == 0),
                        stop=(i == nkc - 1),
                    )
            if bcast == "mm":
                # both heads' denominators -> reciprocals -> selector matmuls
                # broadcast them across partitions (rows 0:64 even, 64:128 odd)
                rtE = rtpool.tile([1, NQ], BF, tag="rt", name=f"rtE_{m}_{j}")
                rtO = rtpool.tile([1, NQ], BF, tag="rt", name=f"rtO_{m}_{j}")
                with nc.allow_low_precision(reason="bf16 recip feeds bcast matmul"):
                    nc.vector.reciprocal(rtE[:], poE[64:65, :])
                    nc.vector.reciprocal(rtO[:], poO[64:65, :])
                rbc_t = psS.tile([P, 2, NQ], F32, tag="psS", name=f"rbc_{m}_{j}")
                rbc = rbc_t[:, 0, :]
                nc.tensor.matmul(rbc, sel[0:1, 0, :], rtE[:], start=True, stop=False)
                nc.tensor.matmul(rbc, sel[0:1, 1, :], rtO[:], start=False, stop=True)
                # stage to SBUF: walrus forbids two PSUM reads in one DVE op
                rbc_sb = bcpool.tile([P, NQ], BF, tag="rbc", name=f"rbcsb_{m}_{j}")
                if (m + j) % 2 == 0:
                    nc.vector.tensor_copy(rbc_sb[:], rbc)
                else:
                    nc.scalar.copy(rbc_sb[:], rbc)
                nc.vector.tensor_tensor(
                    out=attnT[0:64, m, j * NQ : (j + 1) * NQ],
                    in0=poE[0:64, :],
                    in1=rbc_sb[0:64, :],
                    op=mybir.AluOpType.mult,
                )
                st = stpool.tile([64, NQ], BF, tag="st", name=f"st_{m}_{j}")
                nc.vector.tensor_tensor(
                    out=st[:],
                    in0=poO[0:64, :],
                    in1=rbc_sb[64:128, :],
                    op=mybir.AluOpType.mult,
                )
                nc.sync.dma_start(
                    attnT[64:128, m, j * NQ : (j + 1) * NQ], st[:]
                )
                return
            for idx in range(2):
                po = poE if idx == 0 else poO
                if og_stage:
                    og = ogpool.tile([65, NQ], F32, tag="og", name=f"og_{m}_{j}_{idx}")
                    nc.vector.tensor_copy(og[:], po[:])
                else:
                    og = po  # normalize straight out of PSUM
                rt = rtpool.tile([1, NQ], F32, tag="rt", name=f"rt_{m}_{j}_{idx}")
                nc.vector.reciprocal(rt[:], og[64:65, :])
                rbc = bcpool.tile([64, NQ], F32, tag="rbc", name=f"rbc_{m}_{j}_{idx}")
                if pb_bcast:
                    nc.gpsimd.partition_broadcast(rbc[:], rt[:], channels=64)
                else:
                    scr = drpool.tile([NQ], F32, tag="scr", name=f"scr_{m}_{j}_{idx}")
                    nc.sync.dma_start(scr[None, :], rt[:])
                    nc.sync.dma_start(rbc[:], scr[None, :].to_broadcast((64, NQ)))
                if idx == 0:
                    nc.vector.tensor_tensor(
                        out=attnT[0:64, m, j * NQ : (j + 1) * NQ],
                        in0=og[0:64, :],
                        in1=rbc[:],
                        op=mybir.AluOpType.mult,
                    )
                elif st_direct:
                    nc.vector.tensor_tensor(
                        out=attnT[64:128, m, j * NQ : (j + 1) * NQ],
                        in0=og[0:64, :],
                        in1=rbc[:],
                        op=mybir.AluOpType.mult,
                    )
                else:
                    st = stpool.tile([64, NQ], BF, tag="st", name=f"st_{m}_{j}")
                    nc.vector.tensor_tensor(
                        out=st[:],
                        in0=og[0:64, :],
                        in1=rbc[:],
                        op=mybir.AluOpType.mult,
                    )
                    nc.sync.dma_start(
                        attnT[64:128, m, j * NQ : (j + 1) * NQ], st[:]
                    )

        # sweep 1: B1 interleaved with C(j=0) at i-unit granularity
        if phases == "b1only":
            for m in range(H // 2):
                for _ in b1_gen(m):
                    pass
            psb1.release()
            with tc.tile_pool(name="dump", bufs=2) as dp:
                for mm in range(8):
                    t = dp.tile([P, S], F32, tag="o", name=f"dump_{mm}")
                    nc.vector.tensor_copy(t[:], qkT[:, mm, :])
                    nc.sync.dma_start(y_ap[mm * P : (mm + 1) * P, :], t[:])
            cs.close()
            wop.release()
            if use_early_wq:
                wqp0.release()
            top.close()
            continue
        for _ in b1_gen(0):
            pass
        for m in range(H // 2):
            if m + 1 < H // 2:
                fill_q.append(b1_gen(m + 1))
            emit_C(m, 0, fill_per_unit=f1)
        pump(1 << 30)
        psb1.release()

        if phases == "abs1":
            with tc.tile_pool(name="dump", bufs=2) as dp:
                for dd in range(DC):
                    t = dp.tile([P, NQ], F32, tag="o", name=f"dump_{dd}")
                    nc.vector.tensor_copy(t[:], attnT[:, dd, 0:NQ])
                    nc.sync.dma_start(y_ap[dd * P : (dd + 1) * P, 0:NQ], t[:])
            cs.close()
            wop.release()
            if use_early_wq:
                wqp0.release()
            top.close()
            continue

        # D setup
        psY = cs.enter_context(tc.tile_pool(name="psY", bufs=psY_bufs, space="PSUM"))
        ypool = cs.enter_context(tc.tile_pool(name="yp", bufs=3))
        boutbc = ypool.tile([P, D], F32, name="boutbc", tag="boutbc")
        nc.sync.dma_start(boutbc[:], bout_ap[None, :].to_broadcast((P, D)))

        def d_gen(qc):
            for nqq in range(D // NQ):
                ps = psY.tile([P, NQ], F32, tag="psY", name=f"psY_{qc}_{nqq}")
                for kc0 in range(0, DC, 2):
                    for kc in (kc0, kc0 + 1):
                        nc.tensor.matmul(
                            ps[:],
                            attnT[:, kc, qc * P : (qc + 1) * P],
                            wo_half[nqq][:, kc, :],
                            start=(kc == 0),
                            stop=(kc == DC - 1),
                        )
                    yield
                yt = ypool.tile([P, NQ], F32, tag="y", name=f"y_{qc}_{nqq}")
                nc.vector.tensor_tensor(
                    out=yt[:],
                    in0=ps[:],
                    in1=boutbc[:, nqq * NQ : (nqq + 1) * NQ],
                    op=mybir.AluOpType.add,
                )
                nc.sync.dma_start(
                    y_ap[qc * P : (qc + 1) * P, nqq * NQ : (nqq + 1) * NQ], yt[:]
                )

        # sweep 2: C(j=1) with D interleaved as PE filler. D(qc<4) only needs
        # sweep-1 results (attnT query cols 0:512), so its generators queue up
        # from the first C on; qc>=4 drains after the last C.
        nqc = 0
        for m in range(H // 2):
            if m in d_after:
                fill_q.append(d_gen(nqc))
                nqc += 1
            emit_C(m, 1, fill_per_unit=f2)
        for qc in range(nqc, SC):
            fill_q.append(d_gen(qc))
        pump(1 << 30)

        cs.close()
        wop.release()
        if use_early_wq:
            wqp0.release()
        top.close()

      if persist_x:
          xpool_p.release()
          xT_pool.release()

    nc.compile()
    return nc


_CACHED = {}


def _get_nc():
    if "nc" not in _CACHED:
        _CACHED["nc"] = build_kernel(niter=1)
    return _CACHED["nc"]


def kernel(x, W_qkv, b_qkv, W_out, b_out):
    x = np.ascontiguousarray(np.asarray(x, dtype=np.float32))
    W_qkv = np.ascontiguousarray(np.asarray(W_qkv, dtype=np.float32))
    b_qkv = np.ascontiguousarray(np.asarray(b_qkv, dtype=np.float32))
    W_out = np.ascontiguousarray(np.asarray(W_out, dtype=np.float32))
    b_out = np.ascontiguousarray(np.asarray(b_out, dtype=np.float32))
    B = x.shape[0]
    assert x.shape == (8, S, D), f"expected x [8, {S}, {D}], got {x.shape}"

    from concourse.bass_utils import run_bass_kernel_spmd

    nc = _get_nc()
    in_maps = [
        {
            "x": np.ascontiguousarray(x[b]),
            "W_qkv": W_qkv,
            "b_qkv": b_qkv,
            "W_out": W_out,
            "b_out": b_out,
        }
        for b in range(B)
    ]
    res = run_bass_kernel_spmd(nc, in_maps, list(range(B)))
    return np.stack([res.results[b]["y"] for b in range(B)]).astype(np.float32)
